# revision 6
# baseline (speedup 1.0000x reference)
"""CrossAttnBlock kernel for 8 Trainium2 NeuronCores.

Sharding: core c -> (batch b = c//2, token-half s = c%2), 512 query tokens
per core. Cross-attention K/V is computed fully per core (duplicated within
the pair); after cross-attention the per-core residual x2 is exchanged with
one 8-rank AllGather (bf16 payload) so each core rebuilds the partner
half's self-attn K/V locally (attention is permutation-invariant over KV
tokens, so own tokens always sit at positions 0:512).

Performance notes vs the first working version (1316 us):
- Weights are pre-tiled on the host to [128, NT, C, 128] (partition-major
  tiles) so each weight-tile DMA moves >=2KB contiguous per partition
  instead of 512B fragments. DMA descriptor fragmentation was the
  bottleneck (16 engines at ~5 GB/s each).
- Weights and activations stream as bfloat16 (tolerance is 2e-2; measured
  error stays ~1e-3). Residual stream and LN stats stay fp32.
- K/V stay resident in SBUF (no DRAM spill round trips).
- LN scale/bias are folded into the following projection weights on the
  host, so the device LN is just (x - mean) * rsqrt(var + eps).
- V bias is applied with a K=1 accumulating matmul (ones x bias-row);
  softmax 1/Z broadcast uses a K=1 matmul + ACT copy instead of a DRAM
  broadcast round trip.
- Score matmuls for head pairs use row-packed PE tiles (base partitions
  0 and 64 -> concurrent 64-row matmuls).
"""
import sys

sys.path.insert(0, '/opt/trn_rl_repo')

import numpy as np
import ml_dtypes
import concourse.bass as bass
from concourse import bacc
import concourse.tile as tile
from concourse import mybir

F32R = mybir.dt.float32r
F32 = mybir.dt.float32
BF = mybir.dt.bfloat16
AF = mybir.ActivationFunctionType
OP = mybir.AluOpType

N_CORES = 8
B, NSEQ, D, H, HD = 4, 1024, 1024, 16, 64
T = 512            # tokens owned per core
TF = 1024          # full token count per batch
C8 = D // 128      # feature chunks
SCALE = 1.0 / float(np.sqrt(np.float32(HD)))
EPS = 1e-6

_PROGRAM_CACHE = {}


def _build_program():
    nc = bacc.Bacc("TRN2", target_bir_lowering=False, debug=False,
                   num_devices=N_CORES)

    dp = {}
    dp["xT"] = nc.declare_dram_parameter("xT", [128, C8, T], F32R,
                                         isOutput=False)
    dp["kvT"] = nc.declare_dram_parameter("kvT", [128, C8, TF], BF,
                                          isOutput=False)
    # tiled weights [128, NT, C, 128]
    for nm, ntile, cch in [("wq", C8, C8), ("wkv_k", C8, C8),
                           ("wqkv_q", C8, C8), ("wqkv_k", C8, C8),
                           ("wco", C8, C8), ("wso", C8, C8),
                           ("w1", 32, C8), ("w2", C8, 32)]:
        dp[nm] = nc.declare_dram_parameter(nm, [128, ntile, cch, 128], BF,
                                           isOutput=False)
    # V-projection weights kept [128, C, Dout] (moving operand layout)
    dp["wkv_v"] = nc.declare_dram_parameter("wkv_v", [128, C8, D], BF,
                                            isOutput=False)
    dp["wqkv_v"] = nc.declare_dram_parameter("wqkv_v", [128, C8, D], BF,
                                             isOutput=False)
    # biases (LN biases already folded in on host)
    for nm, n in [("bq", D), ("bk", D), ("bco", D), ("bq2", D), ("bk2", D),
                  ("bso", D), ("b2", D), ("b1m", 4 * D)]:
        dp[nm] = nc.declare_dram_parameter(nm, [1, n], F32, isOutput=False)
    dp["bv_row"] = nc.declare_dram_parameter("bv_row", [1, D], BF,
                                             isOutput=False)
    dp["bv2_row"] = nc.declare_dram_parameter("bv2_row", [1, D], BF,
                                              isOutput=False)
    dp["ones"] = nc.declare_dram_parameter("ones", [128, 128], F32R,
                                           isOutput=False)
    dp["ones_bf"] = nc.declare_dram_parameter("ones_bf", [128, 128], BF,
                                              isOutput=False)
    dp["outT"] = nc.declare_dram_parameter("outT", [128, C8, T], F32,
                                           isOutput=True)

    with tile.TileContext(nc) as tc:
        _emit(nc, tc, dp)
    nc.compile()
    return nc


def _emit(nc, tc, dp):
    import contextlib

    ctx = contextlib.ExitStack()
    with ctx:
        consts = ctx.enter_context(tc.tile_pool(name="consts", bufs=1))
        outer = ctx.enter_context(tc.tile_pool(name="outer", bufs=1))
        small = ctx.enter_context(tc.tile_pool(name="small", bufs=1))
        dramp = ctx.enter_context(tc.tile_pool(name="dramp", bufs=1,
                                               space="DRAM"))

        # ---------- constants ----------
        ones_sb = consts.tile([128, 128], F32R)
        nc.sync.dma_start(out=ones_sb[:], in_=dp["ones"][:])
        ones_bf = consts.tile([128, 128], BF)
        nc.sync.dma_start(out=ones_bf[:], in_=dp["ones_bf"][:])
        ones_col = ones_sb[:, 0:1]
        ones_row = ones_sb[0:1, :]
        onesb_col = ones_bf[:, 0:1]
        eps_t = consts.tile([1, 1], F32)
        nc.vector.memset(eps_t[:], EPS)

        def load_col(name, nchunk):
            col = consts.tile([128, nchunk], F32, name=f"col_{name}")
            nc.sync.dma_start(
                out=col[:], in_=dp[name].rearrange("o (c p) -> p (o c)", p=128))
            return col

        bq_c = load_col("bq", C8)
        bk_c = load_col("bk", C8)
        bco_c = load_col("bco", C8)
        bq2_c = load_col("bq2", C8)
        bk2_c = load_col("bk2", C8)
        bso_c = load_col("bso", C8)
        b2_c = load_col("b2", C8)
        b1_c = load_col("b1m", 32)
        bv_sb = consts.tile([1, D], BF, name="bv_sb")
        nc.sync.dma_start(out=bv_sb[:], in_=dp["bv_row"][:])
        bv2_sb = consts.tile([1, D], BF, name="bv2_sb")
        nc.sync.dma_start(out=bv2_sb[:], in_=dp["bv2_row"][:])

        pid = nc.sync.partition_id()
        partner = (pid // 2) * 2 + (1 - pid % 2)

        # ---------- DRAM intermediates ----------
        x2d = dramp.tile([128, C8 * T], BF, name="x2d")
        ag_out = dramp.tile([N_CORES, 128, C8 * T], BF,
                            addr_space="Shared", name="ag_out")

        # residual stream (fp32, SBUF-resident the whole kernel)
        x1 = outer.tile([128, C8, T], F32R, name="x1")
        nc.sync.dma_start(out=x1[:], in_=dp["xT"][:])
        x2 = outer.tile([128, C8, T], F32R, name="x2")
        x3 = outer.tile([128, C8, T], F32R, name="x3")

        # ---------- helpers ----------
        def layer_norm(src_fn, dtype, o_col, dst, dst_sl, pool, pp):
            """LN over the feature axis for 512 tokens; affine folded into
            the next projection on host. src_fn(c) -> [128, 512] AP."""
            stx = pp.tile([1, 512], F32, tag="stx", bufs=1, name="stx")
            stq = pp.tile([1, 512], F32, tag="stq", bufs=1, name="stq")
            for c in range(C8):
                xc = src_fn(c)
                sq = pool.tile([128, 512], dtype, tag="sq", bufs=2, name="sq")
                nc.scalar.activation(out=sq[:], in_=xc, func=AF.Square)
                nc.tensor.matmul(stx[:], o_col, xc,
                                 start=(c == 0), stop=(c == C8 - 1),
                                 skip_group_check=True)
                nc.tensor.matmul(stq[:], o_col, sq[:],
                                 start=(c == 0), stop=(c == C8 - 1),
                                 skip_group_check=True)
            mean = small.tile([1, 512], F32, tag="mean", bufs=2, name="mean")
            nc.vector.tensor_scalar_mul(mean[:], stx[:], 1.0 / D)
            var = small.tile([1, 512], F32, tag="var", bufs=2, name="var")
            nc.vector.tensor_scalar_mul(var[:], stq[:], 1.0 / D)
            m2 = small.tile([1, 512], F32, tag="m2", bufs=2, name="m2")
            nc.vector.tensor_mul(m2[:], mean[:], mean[:])
            nc.vector.tensor_sub(var[:], var[:], m2[:])
            std = small.tile([1, 512], F32, tag="std", bufs=2, name="std")
            nc.scalar.activation(out=std[:], in_=var[:], func=AF.Sqrt,
                                 bias=eps_t[:])
            inv = small.tile([1, 512], F32R, tag="inv", bufs=2, name="inv")
            with nc.allow_low_precision(reason="float32r is 32-bit"):
                nc.vector.reciprocal(inv[:], std[:])
            negminv = small.tile([1, 512], F32R, tag="negminv", bufs=2,
                                 name="negminv")
            nc.vector.scalar_tensor_tensor(
                out=negminv[:], in0=mean[:], scalar=-1.0, in1=inv[:],
                op0=OP.mult, op1=OP.mult)
            a0 = pp.tile([128, 512], F32, tag="bc", bufs=2, name="a0")
            nc.tensor.matmul(a0[:], ones_row, inv[:], start=True, stop=True)
            c0 = pp.tile([128, 512], F32, tag="bc", bufs=2, name="c0")
            nc.tensor.matmul(c0[:], ones_row, negminv[:], start=True,
                             stop=True)
            for c in range(C8):
                xc = src_fn(c)
                nc.vector.tensor_mul(dst[:, c, dst_sl], xc, a0[:])
                nc.vector.tensor_add(dst[:, c, dst_sl], dst[:, c, dst_sl],
                                     c0[:])

        def gemm(w_dram, pool, pp, rhs_list, evict, nts=None, cch=C8):
            """Weight-stationary GEMM streaming pre-tiled weight tiles.
            rhs_list: [(rhs_fn(c) -> [128,512] bf16 AP, key)]."""
            if nts is None:
                nts = range(w_dram.shape[1])
            for nt in nts:
                wt = pool.tile([128, cch, 128], BF, tag=f"wt{cch}", bufs=3,
                               name="wt")
                nc.sync.dma_start(out=wt[:], in_=w_dram[:, nt, :, :])
                for rhs_fn, key in rhs_list:
                    ps = pp.tile([128, 512], F32, tag="mm", bufs=4, name="ps")
                    for c in range(cch):
                        nc.tensor.matmul(ps[:], wt[:, c, :], rhs_fn(c),
                                         start=(c == 0), stop=(c == cch - 1))
                    evict(nt, key, ps)

        def build_v(src_fn, wv_dram, bvrow, jts, v_sb, jt_off, pool, pp):
            """Token-major V (+bias) for 128-token j-tiles into SBUF.
            src_fn(c, sl) -> [128, 128] bf16 token-chunk AP."""
            for dvh in range(2):
                wvh = pool.tile([128, C8, 512], BF, tag="wvh", bufs=2,
                                name="wvh")
                nc.sync.dma_start(out=wvh[:],
                                  in_=wv_dram[:, :, dvh * 512:(dvh + 1) * 512])
                for jt in jts:
                    sl = slice((jt - jt_off) * 128, (jt - jt_off + 1) * 128)
                    ps = pp.tile([128, 512], F32, tag="mm", bufs=4, name="vps")
                    for c in range(C8):
                        nc.tensor.matmul(ps[:], src_fn(c, sl), wvh[:, c, :],
                                         start=(c == 0), stop=False)
                    nc.tensor.matmul(ps[:], ones_bf[0:1, 0:128],
                                     bvrow[:, dvh * 512:(dvh + 1) * 512],
                                     start=False, stop=True)
                    nc.vector.tensor_copy(
                        out=v_sb[:, jt, dvh * 8:(dvh + 1) * 8, 0:64],
                        in_=ps[:].rearrange("p (h e) -> p h e", h=8))

        def attention(qT, kT_sb, v_sb, oT, pool, pp):
            for hp in range(C8):
                o_ps = [pp.tile([65, 512], F32, tag="ops", bufs=4,
                                name=f"ops{i}") for i in (0, 1)]
                for jt in range(8):
                    for i in (0, 1):
                        off = i * 64
                        s_ps = pp.tile([128, 512], F32, tag="sps", bufs=3,
                                       name="sps")
                        nc.tensor.matmul(
                            s_ps[:],
                            kT_sb[off:off + 64, hp, jt * 128:(jt + 1) * 128],
                            qT[off:off + 64, hp, :], start=True, stop=True)
                        pt = pool.tile([128, 512], BF, tag="pt", bufs=4,
                                       name="pt")
                        nc.scalar.activation(out=pt[:], in_=s_ps[:],
                                             func=AF.Exp, scale=SCALE)
                        nc.tensor.matmul(o_ps[i][:], v_sb[:, jt, 2 * hp + i, :],
                                         pt[:], start=(jt == 0), stop=(jt == 7),
                                         skip_group_check=True)
                for i in (0, 1):
                    zrec = small.tile([1, 512], F32R, tag="zrec", bufs=2,
                                      name="zrec")
                    with nc.allow_low_precision(reason="float32r is 32-bit"):
                        nc.vector.reciprocal(zrec[:], o_ps[i][64:65, :])
                    zb_ps = pp.tile([64, 512], F32, tag="zb", bufs=1,
                                    name="zb_ps")
                    nc.tensor.matmul(zb_ps[:], ones_sb[0:1, 0:64], zrec[:],
                                     start=True, stop=True)
                    zb = pool.tile([64, 512], F32R, tag="zbs", bufs=2,
                                   name="zbs")
                    nc.scalar.activation(out=zb[:], in_=zb_ps[:], func=AF.Copy)
                    nc.vector.tensor_mul(oT[i * 64:(i + 1) * 64, hp, :],
                                         o_ps[i][0:64, :], zb[:])

        # ================= Phase 1: cross-attention =================
        with tc.tile_pool(name="p1", bufs=1) as p1:
            qT1 = p1.tile([128, C8, T], BF, name="qT1")
            kT1 = p1.tile([128, C8, TF], BF, name="kT1")
            v1 = p1.tile([128, 8, H, 65], BF, name="v1")
            nc.vector.memset(v1[:, :, :, 64], 1.0)
            oT1 = p1.tile([128, C8, T], BF, name="oT1")

            kv_sb = p1.tile([128, C8, TF], BF, name="kv_sb")
            nc.sync.dma_start(out=kv_sb[:], in_=dp["kvT"][:])
            q_in = p1.tile([128, C8, T], BF, name="q_in")

            with tc.tile_pool(name="ppA", bufs=1, space="PSUM") as ppA:
                layer_norm(lambda c: x1[:, c, :], F32R, ones_col, q_in,
                           slice(0, T), p1, ppA)

                def ev_qT(nt, key, ps):
                    nc.vector.tensor_scalar_add(qT1[:, nt, :], ps[:],
                                                scalar1=bq_c[:, nt:nt + 1])

                gemm(dp["wq"], p1, ppA, [(lambda c: q_in[:, c, :], 0)], ev_qT)

                # kv is normalized in place (stats pass completes before the
                # normalize pass writes; tile tracks the WAR dependency)
                for th in range(2):
                    sl = slice(th * T, (th + 1) * T)
                    layer_norm(lambda c, sl=sl: kv_sb[:, c, sl], BF, onesb_col,
                               kv_sb, sl, p1, ppA)

                def ev_kT(nt, th, ps):
                    nc.vector.tensor_scalar_add(
                        kT1[:, nt, th * T:(th + 1) * T], ps[:],
                        scalar1=bk_c[:, nt:nt + 1])

                gemm(dp["wkv_k"], p1, ppA,
                     [(lambda c, th=th: kv_sb[:, c, th * T:(th + 1) * T], th)
                      for th in range(2)], ev_kT)

                build_v(lambda c, sl: kv_sb[:, c, sl], dp["wkv_v"], bv_sb,
                        range(8), v1, 0, p1, ppA)

            with tc.tile_pool(name="ppB", bufs=1, space="PSUM") as ppB:
                attention(qT1, kT1, v1, oT1, p1, ppB)

            with tc.tile_pool(name="ppC", bufs=1, space="PSUM") as ppC:
                x2d_r = x2d[:].rearrange("p (c t) -> p c t", c=C8)

                def ev_x2(nt, key, ps):
                    nc.vector.scalar_tensor_tensor(
                        out=x2[:, nt, :], in0=ps[:],
                        scalar=bco_c[:, nt:nt + 1], in1=x1[:, nt, :],
                        op0=OP.add, op1=OP.add)
                    x2c = p1.tile([128, 512], BF, tag="x2c", bufs=2,
                                  name="x2c")
                    nc.vector.tensor_copy(out=x2c[:], in_=x2[:, nt, :])
                    nc.sync.dma_start(out=x2d_r[:, nt, :], in_=x2c[:])

                gemm(dp["wco"], p1, ppC, [(lambda c: oT1[:, c, :], 0)], ev_x2)

        # ================= x2 exchange (8-rank AllGather) =================
        nc.gpsimd.collective_compute(
            "AllGather", OP.bypass,
            ins=[x2d[:]],
            outs=[ag_out[:]],
            replica_groups=[list(range(N_CORES))])
        ag_rem = ag_out[bass.ds(partner, 1), :, :].rearrange(
            "o p (c t) -> p (o c) t", c=C8)

        # ================= Phase 2: self-attention =================
        with tc.tile_pool(name="p2", bufs=1) as p2:
            qT2 = p2.tile([128, C8, T], BF, name="qT2")
            kT2 = p2.tile([128, C8, TF], BF, name="kT2")
            v2 = p2.tile([128, 8, H, 65], BF, name="v2")
            nc.vector.memset(v2[:, :, :, 64], 1.0)
            oT2 = p2.tile([128, C8, T], BF, name="oT2")
            s_own = p2.tile([128, C8, T], BF, name="s_own")
            s_rem = p2.tile([128, C8, T], BF, name="s_rem")

            with tc.tile_pool(name="ppC2", bufs=1, space="PSUM") as ppC2:
                # own half (independent of the AllGather)
                layer_norm(lambda c: x2[:, c, :], F32R, ones_col, s_own,
                           slice(0, T), p2, ppC2)

                def ev_qT2(nt, key, ps):
                    nc.vector.tensor_scalar_add(qT2[:, nt, :], ps[:],
                                                scalar1=bq2_c[:, nt:nt + 1])

                gemm(dp["wqkv_q"], p2, ppC2, [(lambda c: s_own[:, c, :], 0)],
                     ev_qT2)

                def ev_kT2(nt, half, ps):
                    nc.vector.tensor_scalar_add(
                        kT2[:, nt, half * T:(half + 1) * T], ps[:],
                        scalar1=bk2_c[:, nt:nt + 1])

                gemm(dp["wqkv_k"], p2, ppC2,
                     [(lambda c: s_own[:, c, :], 0)], ev_kT2)
                build_v(lambda c, sl: s_own[:, c, sl], dp["wqkv_v"], bv2_sb,
                        range(4), v2, 0, p2, ppC2)

                # remote half (depends on the AllGather)
                x2r = p2.tile([128, C8, T], BF, name="x2r")
                nc.sync.dma_start(out=x2r[:], in_=ag_rem)
                layer_norm(lambda c: x2r[:, c, :], BF, onesb_col, s_rem,
                           slice(0, T), p2, ppC2)
                gemm(dp["wqkv_k"], p2, ppC2,
                     [(lambda c: s_rem[:, c, :], 1)], ev_kT2)
                build_v(lambda c, sl: s_rem[:, c, sl], dp["wqkv_v"], bv2_sb,
                        range(4, 8), v2, 4, p2, ppC2)

            with tc.tile_pool(name="ppD", bufs=1, space="PSUM") as ppD:
                attention(qT2, kT2, v2, oT2, p2, ppD)

            with tc.tile_pool(name="ppE", bufs=1, space="PSUM") as ppE:
                def ev_x3(nt, key, ps):
                    nc.vector.scalar_tensor_tensor(
                        out=x3[:, nt, :], in0=ps[:],
                        scalar=bso_c[:, nt:nt + 1], in1=x2[:, nt, :],
                        op0=OP.add, op1=OP.add)

                gemm(dp["wso"], p2, ppE, [(lambda c: oT2[:, c, :], 0)], ev_x3)

        # ================= Phase 3: MLP =================
        with tc.tile_pool(name="p3", bufs=1) as p3:
            m_in = p3.tile([128, C8, T], BF, name="m_in")
            hT = p3.tile([128, 32, T], BF, name="hT")

            with tc.tile_pool(name="ppF", bufs=1, space="PSUM") as ppF:
                layer_norm(lambda c: x3[:, c, :], F32R, ones_col, m_in,
                           slice(0, T), p3, ppF)

                def ev_h(ht, key, ps):
                    nc.scalar.activation(out=hT[:, ht, :], in_=ps[:],
                                         func=AF.Gelu_apprx_tanh,
                                         bias=b1_c[:, ht:ht + 1], scale=1.0)

                gemm(dp["w1"], p3, ppF, [(lambda c: m_in[:, c, :], 0)], ev_h)

                def ev_out(nt, key, ps):
                    ot = p3.tile([128, 512], F32, tag="ot", bufs=2, name="ot")
                    nc.vector.tensor_scalar_add(ot[:], ps[:],
                                                scalar1=b2_c[:, nt:nt + 1])
                    nc.sync.dma_start(out=dp["outT"][:, nt, :], in_=ot[:])

                gemm(dp["w2"], p3, ppF, [(lambda c: hT[:, c, :], 0)], ev_out,
                     cch=32)


def _get_program():
    if "nc" not in _PROGRAM_CACHE:
        _PROGRAM_CACHE["nc"] = _build_program()
    return _PROGRAM_CACHE["nc"]


def _tile_w(w):
    """[Din, Dout] fp32 -> [128, NT, C, 128] bf16 (partition-major tiles)."""
    din, dout = w.shape
    c, nt = din // 128, dout // 128
    return np.ascontiguousarray(
        w.reshape(c, 128, nt, 128).transpose(1, 2, 0, 3)).astype(
            ml_dtypes.bfloat16)


def _vmaj_w(w):
    """[Din, Dout] fp32 -> [128, C, Dout] bf16 (moving-operand layout)."""
    din, dout = w.shape
    c = din // 128
    return np.ascontiguousarray(
        w.reshape(c, 128, dout).transpose(1, 0, 2)).astype(ml_dtypes.bfloat16)


def kernel(**inputs) -> np.ndarray:
    from concourse.bass_utils import run_bass_kernel_spmd

    nc = _get_program()

    f32 = lambda a: np.asarray(a, np.float32)
    x = f32(inputs["x"])
    key_val = f32(inputs["key_val"])

    # fold LN affine params into the following projections
    Wq = f32(inputs["Wq"]) * f32(inputs["ln1_s"])[:, None]
    bq = f32(inputs["ln1_b"]) @ f32(inputs["Wq"])
    Wkv = f32(inputs["Wkv"]) * f32(inputs["ln2_s"])[:, None]
    bkv = f32(inputs["ln2_b"]) @ f32(inputs["Wkv"])
    Wqkv = f32(inputs["Wqkv"]) * f32(inputs["ln3_s"])[:, None]
    bqkv = f32(inputs["ln3_b"]) @ f32(inputs["Wqkv"])
    W1 = f32(inputs["W1"]) * f32(inputs["ln4_s"])[:, None]
    b1m = f32(inputs["b1"]) + f32(inputs["ln4_b"]) @ f32(inputs["W1"])

    shared = {
        "wq": _tile_w(Wq),
        "wkv_k": _tile_w(Wkv[:, :D]),
        "wkv_v": _vmaj_w(Wkv[:, D:]),
        "wqkv_q": _tile_w(Wqkv[:, :D]),
        "wqkv_k": _tile_w(Wqkv[:, D:2 * D]),
        "wqkv_v": _vmaj_w(Wqkv[:, 2 * D:]),
        "wco": _tile_w(f32(inputs["Wco"])),
        "wso": _tile_w(f32(inputs["Wso"])),
        "w1": _tile_w(W1),
        "w2": _tile_w(f32(inputs["W2"])),
        "bq": bq[None, :], "bk": bkv[None, :D],
        "bv_row": bkv[None, D:].astype(ml_dtypes.bfloat16),
        "bq2": bqkv[None, :D], "bk2": bqkv[None, D:2 * D],
        "bv2_row": bqkv[None, 2 * D:].astype(ml_dtypes.bfloat16),
        "bco": f32(inputs["bco"])[None, :],
        "bso": f32(inputs["bso"])[None, :],
        "b1m": b1m[None, :],
        "b2": f32(inputs["b2"])[None, :],
        "ones": np.ones((128, 128), np.float32),
        "ones_bf": np.ones((128, 128), ml_dtypes.bfloat16),
    }
    in_maps = []
    for c in range(N_CORES):
        b, s = c // 2, c % 2
        m = dict(shared)
        xTc = x[b, s * T:(s + 1) * T, :].T  # [D, T]
        m["xT"] = np.ascontiguousarray(
            xTc.reshape(C8, 128, T).transpose(1, 0, 2))
        kvTc = key_val[b].T  # [D, TF]
        m["kvT"] = np.ascontiguousarray(
            kvTc.reshape(C8, 128, TF).transpose(1, 0, 2)).astype(
                ml_dtypes.bfloat16)
        in_maps.append(m)

    res = run_bass_kernel_spmd(nc, in_maps, list(range(N_CORES)))
    _PROGRAM_CACHE["last_result"] = res

    out = np.empty((B, NSEQ, D), np.float32)
    for c in range(N_CORES):
        b, s = c // 2, c % 2
        o = np.asarray(res.results[c]["outT"], np.float32)  # [128, C8, T]
        out[b, s * T:(s + 1) * T, :] = o.transpose(2, 1, 0).reshape(T, D)
    return out
